# revision 4
# baseline (speedup 1.0000x reference)
"""KANLinear Trainium2 kernel.

Strategy:
  - Uniform-knot cubic B-spline basis via two-sided truncated powers:
      basis_j(x) = (1/6) sum_r a_r * relu(y - j - r)^3        (j = 4..7, "up")
      basis_j(x) = (1/6) sum_r a_r * relu((j+4-r) - y)^3      (j = 0..3, "down")
    with y = (x - t0)/h, a = [1,-4,6,-4,1].  The 8->14 recombination matrix
    is folded into the spline weights on the host, so the spline branch
    becomes a dense matmul with contraction over (cube_range, in_feature).
  - x is clipped to the grid support on the host (exact: all basis functions
    vanish outside), which bounds cube magnitudes and drops 2 cube ranges.
  - Fused with the base branch: out = [silu(x) | cubes] @ [baseW.T | wA],
    total K = 1024 + 14*1024 = 15360.
  - float32r (TF32-like, ~11 explicit mantissa bits) matmuls at full PE rate.
  - Data-parallel over batch: 8 cores x 1024 rows.
  - K-blocked accumulation: PSUM accumulates within a k-block, SBUF fp32
    accumulator across k-blocks, so the 48MB weight tensor streams once.
"""
import numpy as np

P = 128
NCORES = 8
BATCH, IN_F, OUT_F = 8192, 1024, 1024
B_LOC = BATCH // NCORES          # 1024
N_IC = IN_F // P                 # 8 input-feature chunks
N_OC = OUT_F // P                # 8 output chunks
UP_MS = list(range(4, 11))       # 7 up cube ranges
DN_MS = list(range(1, 8))        # 7 down cube ranges
N_RANGE = 1 + len(UP_MS) + len(DN_MS)   # 15 weight ranges (base + cubes)
N_KT = N_RANGE * N_IC            # 120 k-tiles of 128
KB = 10                          # k-tiles per block
N_KB = (N_KT + KB - 1) // KB     # 12
NHALF = B_LOC // 512             # 2 matmul N-chunks of 512

# ACT params per cube range, set by _prep() before _build_nc()
SCALES = [0.0] * 14
BIASES = [0.0] * 14

_BUILT = {}


def _build_nc(repeat=1):
    import concourse.bacc as bacc
    import concourse.mybir as mybir
    from concourse import tile

    AF = mybir.ActivationFunctionType
    ALU = mybir.AluOpType
    F32 = mybir.dt.float32
    F32R = mybir.dt.float32r

    nc = bacc.Bacc("TRN2", target_bir_lowering=False, debug=False)

    x_d = nc.dram_tensor("x", [N_IC, P, B_LOC], F32, kind="ExternalInput")
    xc_d = nc.dram_tensor("xc", [N_IC, P, B_LOC], F32, kind="ExternalInput")
    w_d = nc.dram_tensor("w", [N_KB, N_OC, P, KB * P], F32R, kind="ExternalInput")
    out_d = nc.dram_tensor("out", [N_OC, P, B_LOC], F32, kind="ExternalOutput")

    with tile.TileContext(nc) as tc:
        with (
            tc.tile_pool(name="consts", bufs=1) as cpool,
            tc.tile_pool(name="xcp", bufs=1) as xcp,
            tc.tile_pool(name="xst", bufs=1) as xst,
            tc.tile_pool(name="basis", bufs=1) as bpool,
            tc.tile_pool(name="accp", bufs=1) as accp,
            tc.tile_pool(name="wp", bufs=3) as wp,
            tc.tile_pool(name="tmp", bufs=3) as tmp,
            tc.tile_pool(name="psum", bufs=6, space="PSUM") as pp,
        ):
            bias_c = []
            for ri in range(14):
                bc = cpool.tile([P, 1], F32, name=f"biasc{ri}")
                nc.any.memset(bc[:], float(BIASES[ri]))
                bias_c.append(bc)

            # resident clipped-x tiles
            xc_t = []
            for ic in range(N_IC):
                t = xcp.tile([P, B_LOC], F32, name=f"xc{ic}")
                nc.sync.dma_start(t[:], xc_d[ic])
                xc_t.append(t)

            def emit_body(rep=0):
                acc = [accp.tile([P, B_LOC], F32, name=f"acc{oc}_{rep}",
                                 tag=f"acc{oc}") for oc in range(N_OC)]

                def make_basis(kt, rep=rep, acc=acc):
                    """Allocate+compute basis k-tile [P, B_LOC] in f32r."""
                    rng, ic = divmod(kt, N_IC)
                    bt = bpool.tile([P, B_LOC], F32R, name=f"bt{kt}_{rep}",
                                    tag=f"bt{kt % (2 * KB)}")
                    if rng == 0:
                        xt = xst.tile([P, B_LOC], F32, name=f"xs{ic}_{rep}",
                                      tag=f"xs{ic % 2}")
                        nc.sync.dma_start(xt[:], x_d[ic])
                        nc.scalar.activation(bt[:], xt[:], AF.Silu)
                        return bt
                    ri = rng - 1          # 0..13 cube range index
                    sc = float(SCALES[ri])
                    t_t = tmp.tile([P, B_LOC], F32, name=f"t{kt}_{rep}", tag="t")
                    nc.scalar.activation(t_t[:], xc_t[ic][:], AF.Relu,
                                         bias=bias_c[ri][:], scale=sc)
                    sq = tmp.tile([P, B_LOC], F32, name=f"sq{kt}_{rep}", tag="sq")
                    if (kt % 14) < 10:
                        nc.scalar.activation(sq[:], xc_t[ic][:], AF.Square,
                                             bias=bias_c[ri][:], scale=sc)
                    else:
                        nc.vector.tensor_tensor(sq[:], t_t[:], t_t[:], ALU.mult)
                    nc.vector.scalar_tensor_tensor(bt[:], sq[:], 0.0, t_t[:],
                                                   ALU.bypass, ALU.mult)
                    return bt

                kt0 = 0
                for kb in range(N_KB):
                    nkt = min(KB, N_KT - kt0)
                    btiles = [make_basis(kt0 + i) for i in range(nkt)]
                    for oc in range(N_OC):
                        wt = wp.tile([P, KB * P], F32R, name=f"w{kb}_{oc}_{rep}",
                                     tag="w")
                        nc.sync.dma_start(wt[:], w_d[kb, oc])
                        for bh in range(NHALF):
                            ps = pp.tile([P, 512], F32,
                                         name=f"ps{kb}_{oc}_{bh}_{rep}", tag="ps")
                            for i in range(nkt):
                                nc.tensor.matmul(
                                    ps[:], wt[:, i * P:(i + 1) * P],
                                    btiles[i][:, bh * 512:(bh + 1) * 512],
                                    start=(i == 0), stop=(i == nkt - 1))
                            dst = acc[oc][:, bh * 512:(bh + 1) * 512]
                            if kb == 0:
                                nc.scalar.copy(dst, ps[:])
                            else:
                                nc.vector.tensor_tensor(dst, dst, ps[:], ALU.add)
                    kt0 += nkt

                for oc in range(N_OC):
                    nc.sync.dma_start(out_d[oc], acc[oc][:])

            if repeat == 1:
                emit_body()
            else:
                with tc.For_i(0, repeat, 1):
                    emit_body()

    nc.compile()
    return nc


def _prep(x, grid, base_weight, spline_weight, spline_scaler):
    knots = np.asarray(grid, np.float64)[0]          # [12]
    h = (knots[-1] - knots[0]) / (len(knots) - 1)
    t0 = knots[0]
    inv_h = 1.0 / h

    # ACT params: up m: relu(y - m) = relu(x*inv_h + (-t0/h - m))
    #             dn m: relu(m - y) = relu(-x*inv_h + (t0/h + m))
    scales, biases = [], []
    for m in UP_MS:
        scales.append(inv_h); biases.append(-t0 * inv_h - m)
    for m in DN_MS:
        scales.append(-inv_h); biases.append(t0 * inv_h + m)
    global SCALES, BIASES
    SCALES = scales
    BIASES = biases

    # host x prep: transpose to [in, batch], clip copy
    xT = np.ascontiguousarray(np.asarray(x, np.float32).T)      # [IN, BATCH]
    lo = np.float32(t0 - 0.01 * h)
    hi = np.float32(knots[-1] + 0.01 * h)
    xcT = np.clip(xT, lo, hi)

    # weight folding
    a5 = np.array([1., -4., 6., -4., 1.]) / 6.0
    swsc = (np.asarray(spline_weight, np.float64)
            * np.asarray(spline_scaler, np.float64)[:, None, :])  # [in, 8, out]
    wU = {m: np.zeros((IN_F, OUT_F)) for m in UP_MS}
    wD = {m: np.zeros((IN_F, OUT_F)) for m in DN_MS}
    for j in range(4, 8):
        for r in range(5):
            m = j + r
            if m in wU:
                wU[m] += a5[r] * swsc[:, j, :]
    for j in range(0, 4):
        for r in range(5):
            m = j + 4 - r
            if m in wD:
                wD[m] += a5[r] * swsc[:, j, :]

    # assemble lhsT ranges in k order: base, up(m asc), down(m asc)
    ranges = [np.asarray(base_weight, np.float64).T]             # [in, out]
    ranges += [wU[m] for m in UP_MS]
    ranges += [wD[m] for m in DN_MS]
    wK = np.concatenate(ranges, axis=0).astype(np.float32)       # [15360, out]

    # layout [N_KB, N_OC, P, KB*P]: w_d[kb, oc, p, i*P + q] = wK[(kb*KB+i)*P + p,
    #                                                            oc*P + q]
    wAll = np.zeros((N_KB, N_OC, P, KB * P), np.float32)
    for kb in range(N_KB):
        nkt = min(KB, N_KT - kb * KB)
        blk = wK[kb * KB * P:(kb * KB + nkt) * P]                 # [nkt*P, out]
        blk = blk.reshape(nkt, P, N_OC, P)                        # [i, p, oc, q]
        wAll[kb, :, :, :nkt * P] = np.ascontiguousarray(
            blk.transpose(2, 1, 0, 3)).reshape(N_OC, P, nkt * P)
    return xT, xcT, wAll


def _run(nc, in_maps):
    from concourse.bass_utils import run_bass_kernel_spmd
    return run_bass_kernel_spmd(nc, in_maps, core_ids=list(range(NCORES)))


def kernel(x, grid, base_weight, spline_weight, spline_scaler, _repeat=1):
    xT, xcT, wAll = _prep(x, grid, base_weight, spline_weight, spline_scaler)

    if _repeat not in _BUILT:
        _BUILT[_repeat] = _build_nc(_repeat)
    nc = _BUILT[_repeat]

    in_maps = []
    for c in range(NCORES):
        xs = np.ascontiguousarray(
            xT[:, c * B_LOC:(c + 1) * B_LOC].reshape(N_IC, P, B_LOC))
        xcs = np.ascontiguousarray(
            xcT[:, c * B_LOC:(c + 1) * B_LOC].reshape(N_IC, P, B_LOC))
        in_maps.append({"x": xs, "xc": xcs, "w": wAll})

    res = _run(nc, in_maps)

    out = np.empty((BATCH, OUT_F), np.float32)
    for c in range(NCORES):
        o = res.results[c]["out"].reshape(OUT_F, B_LOC)   # [out, b_loc]
        out[c * B_LOC:(c + 1) * B_LOC, :] = o.T
    return out
